# revision 2
# baseline (speedup 1.0000x reference)
"""Multi-head attention, tensor-parallel across 8 Trainium2 NeuronCores. v2

Sharding: core = (batch b, head-group g), 4 heads per core (DH=256).
Per-core layout is pair-oriented: heads (h0,h1) and (h2,h3) form "pairs";
within a pair, the even head lives on SBUF partitions 0-63 and the odd head
on partitions 64-127.  This enables:
  - row-tiled score matmuls: K=64 matmuls for the two heads of a pair run
    in different PE row-groups (tile_position auto-derived from
    base_partition) and execute concurrently.
  - col-tiled AV matmuls: lhsT = V_h [128j, 64] for the even head targets
    psum partitions 0-63 and the odd head partitions 64-127 of the same
    psum tile; the two K=128/M=64 matmuls run in different PE col-groups
    concurrently at full array utilization.
Softmax Z is NOT computed with a ones-column (that would force M=65);
instead zpart[p, i] += E[p, i] accumulates on the Vector engine per j-tile
and a final ones-vector matmul reduces over partitions.
exp() runs as one N=2048 ACT instruction per (pair, jt) reading a 4-bank
psum score tile [128, 2048] = [h_even 1024-i | h_odd 1024-i].
i is processed in two halves (IH=1024) so AV accumulators fit in 2 psum
banks per pair; out-projection of half 0 overlaps attention of half 1.
"""

import numpy as np

B, S, D, H = 2, 2048, 1024, 16
DK = D // H              # 64
N_CORES = 8
GROUPS = N_CORES // B    # 4 head-groups
DH = D // GROUPS         # 256 per core
H_CORE = DH // DK        # 4 heads per core
SCALE = 1.0 / float(np.sqrt(DK))

P = 128
SC = 512
IH = 1024                # i-half
JT = S // P              # 16 j tiles
KT = D // P              # 8 contraction tiles for projections
NSC = S // SC            # 4
PAIRS = H_CORE // 2      # 2
NOUT = D // P            # 8


def build_nc():
    import concourse.bacc as bacc
    import concourse.mybir as mybir
    import concourse.tile as tile

    f32 = mybir.dt.float32
    bf16 = mybir.dt.bfloat16
    Exp = mybir.ActivationFunctionType.Exp
    cdt = bf16

    nc = bacc.Bacc("TRN2", target_bir_lowering=False, debug=False)

    qT = nc.dram_tensor("qT", [D, S], cdt, kind="ExternalInput")
    kTd = nc.dram_tensor("kTd", [D, S], cdt, kind="ExternalInput")
    vT = nc.dram_tensor("vT", [D, S], cdt, kind="ExternalInput")
    wq = nc.dram_tensor("wq", [D, DH], cdt, kind="ExternalInput")
    wk = nc.dram_tensor("wk", [D, DH], cdt, kind="ExternalInput")
    wv = nc.dram_tensor("wv", [D, DH], cdt, kind="ExternalInput")
    wo = nc.dram_tensor("wo", [DH, D], cdt, kind="ExternalInput")
    bq = nc.dram_tensor("bq", [P, PAIRS], f32, kind="ExternalInput")
    bk = nc.dram_tensor("bk", [P, PAIRS], f32, kind="ExternalInput")
    bvb = nc.dram_tensor("bvb", [P, DH], f32, kind="ExternalInput")
    bo = nc.dram_tensor("bo", [P, NOUT], f32, kind="ExternalInput")
    outT = nc.dram_tensor("outT", [D, S], cdt, kind="ExternalOutput")

    with tile.TileContext(nc) as tc:
        with (
            tc.tile_pool(name="const", bufs=1) as cpool,
            tc.tile_pool(name="pers", bufs=1) as pers,
            tc.tile_pool(name="stream", bufs=1) as stream,
            tc.tile_pool(name="psum", bufs=1, space="PSUM") as psum,
            tc.tile_pool(name="dscratch", bufs=1, space="DRAM") as dscratch,
        ):
            # ---- constants ----
            wq_sb = cpool.tile([P, KT, DH], cdt, name="wq_sb")
            wk_sb = cpool.tile([P, KT, DH], cdt, name="wk_sb")
            wv_sb = cpool.tile([P, KT, DH], cdt, name="wv_sb")
            wo_sb = cpool.tile([P, PAIRS, D], cdt, name="wo_sb")
            bq_sb = cpool.tile([P, PAIRS], f32, name="bq_sb")
            bk_sb = cpool.tile([P, PAIRS], f32, name="bk_sb")
            bvb_sb = cpool.tile([P, DH], f32, name="bvb_sb")
            bo_sb = cpool.tile([P, NOUT], f32, name="bo_sb")
            ones_sb = cpool.tile([P, 1], cdt, name="ones_sb")
            nc.sync.dma_start(wq_sb[:], wq[:, :].rearrange("(ko p) n -> p ko n", p=P))
            nc.sync.dma_start(wk_sb[:], wk[:, :].rearrange("(ko p) n -> p ko n", p=P))
            nc.sync.dma_start(wv_sb[:], wv[:, :].rearrange("(ko p) n -> p ko n", p=P))
            nc.sync.dma_start(wo_sb[:], wo[:, :].rearrange("(c p) n -> p c n", p=P))
            nc.sync.dma_start(bq_sb[:], bq[:, :])
            nc.sync.dma_start(bk_sb[:], bk[:, :])
            nc.sync.dma_start(bvb_sb[:], bvb[:, :])
            nc.sync.dma_start(bo_sb[:], bo[:, :])
            nc.vector.memset(ones_sb[:], 1.0)

            # ---- persistent activations ----
            qt_p = [pers.tile([P, S], cdt, name=f"qt{c}") for c in range(PAIRS)]
            kt_p = [pers.tile([P, S], cdt, name=f"kt{c}") for c in range(PAIRS)]
            v_sb = pers.tile([P, JT, DH], cdt, name="v_sb")
            on_p = [pers.tile([P, S], cdt, name=f"on{c}") for c in range(PAIRS)]
            zp = {(c, ih): pers.tile([P, 2 * IH], cdt, name=f"zp{c}_{ih}")
                  for c in range(PAIRS) for ih in range(2)}

            # ---- projection generators ----
            def qk_proj(src, w_sb, b_sb, dst, c, si_list):
                """Project one pair-chunk c for the given si chunks into the
                pair tile dst[c]: psum rows 0-63 = even head dims, 64-127 odd."""
                for si in si_list:
                    ins = []
                    for kt in range(KT):
                        t = stream.tile([P, SC], cdt, tag="instream", bufs=12,
                                        name=f"in_{src.name}_{c}_{si}_{kt}")
                        nc.gpsimd.dma_start(
                            t[:], src[kt * P:(kt + 1) * P, si * SC:(si + 1) * SC])
                        ins.append(t)
                        yield
                    ps = psum.tile([P, SC], f32, tag="mm", bufs=2,
                                   name=f"ps_{src.name}_{c}_{si}")
                    for kt in range(KT):
                        nc.tensor.matmul(
                            ps[:],
                            lhsT=w_sb[:, kt, c * P:(c + 1) * P],
                            rhs=ins[kt][:],
                            start=(kt == 0), stop=(kt == KT - 1))
                        yield
                    nc.vector.tensor_add(
                        dst[:, si * SC:(si + 1) * SC], ps[:],
                        b_sb[:, c:c + 1].to_broadcast((P, SC)))
                    yield

            def v_proj(si_list):
                """V natural: psum[s_sub, dh] accumulated over kt."""
                for si in si_list:
                    ins = []
                    for kt in range(KT):
                        t = stream.tile([P, SC], cdt, tag="instream", bufs=12,
                                        name=f"in_v_{si}_{kt}")
                        nc.sync.dma_start(
                            t[:], vT[kt * P:(kt + 1) * P, si * SC:(si + 1) * SC])
                        ins.append(t)
                        yield
                    for sub in range(SC // P):
                        jt_idx = si * (SC // P) + sub
                        ps = psum.tile([P, DH], f32, tag="mm", bufs=2,
                                       name=f"ps_v_{jt_idx}")
                        for kt in range(KT):
                            nc.tensor.matmul(
                                ps[:],
                                lhsT=ins[kt][:, sub * P:(sub + 1) * P],
                                rhs=wv_sb[:, kt, :],
                                start=(kt == 0), stop=(kt == KT - 1))
                            yield
                        nc.vector.tensor_add(v_sb[:, jt_idx, :], ps[:], bvb_sb[:])
                        yield

            # ---- attention generator for one (pair, ih) ----
            _avs = {}
            _avsb = {}

            def attention(pair, ih):
                i0 = ih * IH
                avs = [psum.tile([P, SC], f32, tag="av", bufs=2,
                                 name=f"av_{pair}_{ih}_{ic}") for ic in range(2)]
                _avs[(pair, ih)] = avs
                for jt in range(JT):
                    e_hh = []
                    for hh in range(2):   # even / odd head of pair
                        b0 = hh * DK      # base partition 0 or 64
                        sc_t = psum.tile([P, IH], f32, tag="sc", bufs=2,
                                         name=f"sc_{pair}_{ih}_{jt}_{hh}")
                        for ic in range(2):
                            nc.tensor.matmul(
                                sc_t[:, ic * SC:(ic + 1) * SC],
                                lhsT=kt_p[pair][b0:b0 + DK, jt * P:(jt + 1) * P],
                                rhs=qt_p[pair][b0:b0 + DK,
                                               i0 + ic * SC:i0 + (ic + 1) * SC],
                                start=True, stop=True)
                        e_t = stream.tile([P, IH], cdt, tag="e", bufs=6,
                                          name=f"e_{pair}_{ih}_{jt}_{hh}")
                        nc.scalar.activation(e_t[:], sc_t[:], Exp,
                                             bias=0.0, scale=SCALE)
                        e_hh.append(e_t)
                    # AV, col-tiled pair: even head -> psum rows 0-63,
                    # odd head -> rows 64-127
                    for ic in range(2):
                        for hh in range(2):
                            h = pair * 2 + hh
                            nc.tensor.matmul(
                                avs[ic][hh * DK:(hh + 1) * DK, :],
                                lhsT=v_sb[:, jt, h * DK:(h + 1) * DK],
                                rhs=e_hh[hh][:, ic * SC:(ic + 1) * SC],
                                start=(jt == 0), stop=(jt == JT - 1))
                    # zpart accumulate on DVE (after AV so AV is not gated)
                    for hh in range(2):
                        if jt == 0:
                            nc.vector.tensor_copy(
                                zp[pair, ih][:, hh * IH:(hh + 1) * IH], e_hh[hh][:])
                        else:
                            nc.vector.tensor_add(
                                zp[pair, ih][:, hh * IH:(hh + 1) * IH],
                                zp[pair, ih][:, hh * IH:(hh + 1) * IH], e_hh[hh][:])
                    yield
                # fast psum release: copy AV accumulators to SBUF f32
                av_sb = stream.tile([P, IH], f32, tag="avsb", bufs=2,
                                    name=f"avsb_{pair}_{ih}")
                for ic in range(2):
                    nc.vector.tensor_copy(av_sb[:, ic * SC:(ic + 1) * SC],
                                          avs[ic][:])
                _avsb[(pair, ih)] = av_sb
                yield

            def attention_fin(pair, ih):
                i0 = ih * IH
                av_sb = _avsb[(pair, ih)]
                # ---- Z: partition-reduce zpart with ones-matmuls ----
                # 4 chunks of 512 -> rows 0,32,64,96 of one psum bank
                zps = psum.tile([P, SC], f32, tag="mm", bufs=2,
                                name=f"zps_{pair}_{ih}")
                for q in range(2 * IH // SC):
                    nc.tensor.matmul(
                        zps[q * 32:q * 32 + 1, :],
                        lhsT=ones_sb[:],
                        rhs=zp[pair, ih][:, q * SC:(q + 1) * SC],
                        start=True, stop=True,
                        tile_position=(0, q * 32))
                    yield
                # reshape z rows [4x512] into [16,128] via DRAM to give the
                # reciprocal 16 lanes of work instead of 1
                zraw = stream.tile([P, SC], f32, tag="zraw", bufs=2,
                                   name=f"zraw_{pair}_{ih}")
                zd_raw = dscratch.tile([4, SC], f32, tag="zdr", bufs=2,
                                       name=f"zdr_{pair}_{ih}")
                for q in range(4):
                    nc.vector.tensor_copy(zraw[q * 32:q * 32 + 1, :],
                                          zps[q * 32:q * 32 + 1, :])
                    nc.sync.dma_start(zd_raw[q:q + 1, :],
                                      zraw[q * 32:q * 32 + 1, :])
                zr = stream.tile([16, P], f32, tag="zr", bufs=2,
                                 name=f"zr_{pair}_{ih}")
                nc.sync.dma_start(
                    zr[:], zd_raw[:, :].rearrange("q (a b) -> (q a) b", b=P))
                rz = stream.tile([16, P], f32, tag="rz", bufs=2,
                                 name=f"rz_{pair}_{ih}")
                nc.vector.reciprocal(rz[:], zr[:])
                zd = dscratch.tile([2, IH], f32, tag="zd", bufs=2,
                                   name=f"zd_{pair}_{ih}")
                for hh in range(2):
                    nc.sync.dma_start(
                        zd[hh:hh + 1, :].rearrange("a (p b) -> (a p) b", b=P),
                        rz[hh * 8:(hh + 1) * 8, :])
                yield
                rzb = stream.tile([P, IH], f32, tag="rzb", bufs=2,
                                  name=f"rzb_{pair}_{ih}")
                for hh in range(2):
                    nc.sync.dma_start(
                        rzb[hh * DK:(hh + 1) * DK, :],
                        zd[hh:hh + 1, :].to_broadcast((DK, IH)))
                # normalize: on_p = av_sb * broadcast(1/Z), bf16
                nc.vector.tensor_mul(on_p[pair][:, i0:i0 + IH], av_sb[:], rzb[:])
                yield

            # ---- output projection for one ih ----
            def oproj(ih):
                i0 = ih * IH
                for n in range(NOUT):
                    for ic in range(2):
                        ps = psum.tile([P, SC], f32, tag="mm", bufs=2,
                                       name=f"ps_o_{ih}_{n}_{ic}")
                        for c in range(PAIRS):
                            nc.tensor.matmul(
                                ps[:],
                                lhsT=wo_sb[:, c, n * P:(n + 1) * P],
                                rhs=on_p[c][:, i0 + ic * SC:i0 + (ic + 1) * SC],
                                start=(c == 0), stop=(c == PAIRS - 1))
                        o_sb = stream.tile([P, SC], cdt, tag="osb", bufs=4,
                                           name=f"o_sb_{ih}_{n}_{ic}")
                        nc.vector.tensor_add(
                            o_sb[:], ps[:],
                            bo_sb[:, n:n + 1].to_broadcast((P, SC)))
                        nc.sync.dma_start(
                            outT[n * P:(n + 1) * P, i0 + ic * SC:i0 + (ic + 1) * SC],
                            o_sb[:])
                        yield

            def run(gen):
                for _ in gen:
                    pass

            def interleave(main, filler, ratio):
                """Step main; after each main step, step filler ratio times."""
                for _ in main:
                    for _ in range(ratio):
                        if filler is not None:
                            if next(filler, StopIteration) is StopIteration:
                                filler = None
                for _ in (filler or ()):
                    pass

            def chain(*gens):
                for g in gens:
                    yield from g

            # Phase 0: minimal prefix for attention(pair0, ih0) jt 0-3
            run(qk_proj(kTd, wk_sb, bk_sb, kt_p[0], 0, (0,)))
            run(qk_proj(qT, wq_sb, bq_sb, qt_p[0], 0, (0, 1)))
            run(v_proj((0,)))

            # P1: attention(pair0, ih0); K-c0/V si chunks feed just ahead of
            # their jt consumption; Q-c0 si23 (needed by P2) fills the rest.
            rest1 = chain(
                qk_proj(kTd, wk_sb, bk_sb, kt_p[0], 0, (1,)),
                v_proj((1,)),
                qk_proj(kTd, wk_sb, bk_sb, kt_p[0], 0, (2,)),
                v_proj((2,)),
                qk_proj(kTd, wk_sb, bk_sb, kt_p[0], 0, (3,)),
                v_proj((3,)),
                qk_proj(qT, wq_sb, bq_sb, qt_p[0], 0, (2, 3)),
            )
            interleave(attention(0, 0), rest1, 20)
            # P2: attention(pair0, ih1); start K/Q for pair1
            rest2 = chain(
                attention_fin(0, 0),
                qk_proj(kTd, wk_sb, bk_sb, kt_p[1], 1, (0,)),
                qk_proj(qT, wq_sb, bq_sb, qt_p[1], 1, (0, 1)),
                qk_proj(kTd, wk_sb, bk_sb, kt_p[1], 1, (1,)),
            )
            interleave(attention(0, 1), rest2, 5)
            # P3: attention(pair1, ih0); K-c1 si2/si3 feed jt8+/jt12+
            rest3 = chain(
                qk_proj(kTd, wk_sb, bk_sb, kt_p[1], 1, (2,)),
                qk_proj(kTd, wk_sb, bk_sb, kt_p[1], 1, (3,)),
                attention_fin(0, 1),
                qk_proj(qT, wq_sb, bq_sb, qt_p[1], 1, (2, 3)),
            )
            interleave(attention(1, 0), rest3, 5)
            # P4: attention(pair1, ih1); pair1-ih0 finalize + oproj(ih0)
            rest4 = chain(attention_fin(1, 0), oproj(0))
            interleave(attention(1, 1), rest4, 2)
            # P5: tail
            run(attention_fin(1, 1))
            run(oproj(1))

    nc.finalize()
    return nc


def make_in_maps(query, key, value, Wq, bq, Wk, bk, Wv, bv, Wo, bo):
    import ml_dtypes
    f = lambda a: np.asarray(a, dtype=np.float32)
    cvt = lambda a: np.ascontiguousarray(
        np.asarray(a, np.float32).astype(ml_dtypes.bfloat16))
    query, key, value = f(query), f(key), f(value)
    Wq, Wk, Wv, Wo = f(Wq), f(Wk), f(Wv), f(Wo)
    bq, bk, bv, bo = f(bq), f(bk), f(bv), f(bo)
    in_maps = []
    for core in range(N_CORES):
        b, g = core // GROUPS, core % GROUPS
        sl = slice(g * DH, (g + 1) * DH)
        in_maps.append({
            "qT": cvt(query[b].T),
            "kTd": cvt(key[b].T),
            "vT": cvt(value[b].T),
            "wq": cvt(Wq[:, sl]),
            "wk": cvt(Wk[:, sl]),
            "wv": cvt(Wv[:, sl]),
            "wo": cvt(Wo[sl, :]),
            # pair-chunk bias layout: [128, PAIRS]; partition p of chunk c is
            # head-dim p of pair c (rows 0-63 even head, 64-127 odd head)
            "bq": np.ascontiguousarray(bq[sl].reshape(PAIRS, P).T),
            "bk": np.ascontiguousarray(bk[sl].reshape(PAIRS, P).T),
            "bvb": np.ascontiguousarray(
                np.broadcast_to(bv[sl][None], (P, DH))),
            "bo": (np.ascontiguousarray(bo.reshape(NOUT, P).T)
                   if g == 0 else np.zeros((P, NOUT), np.float32)),
        })
    return in_maps


TRACE = False
LAST_RESULT = None
DTYPE = "bf16"
_NC_CACHE = {}


def kernel(query, key, value, Wq, bq, Wk, bk, Wv, bv, Wo, bo):
    global LAST_RESULT
    from concourse.bass_utils import run_bass_kernel_spmd

    if "nc" not in _NC_CACHE:
        _NC_CACHE["nc"] = build_nc()
    nc = _NC_CACHE["nc"]

    in_maps = make_in_maps(query, key, value, Wq, bq, Wk, bk, Wv, bv, Wo, bo)
    kwargs = {}
    if TRACE:
        kwargs = dict(trace=True, trace_cores=[0])
    res = run_bass_kernel_spmd(nc, in_maps, core_ids=list(range(N_CORES)), **kwargs)
    LAST_RESULT = res

    out = np.zeros((B, S, D), np.float32)
    for core in range(N_CORES):
        b = core // GROUPS
        out[b] += res.results[core]["outT"].T.astype(np.float32)
    return out


# revision 3
# speedup vs baseline: 1.0004x; 1.0004x over previous
"""Multi-head attention, tensor-parallel across 8 Trainium2 NeuronCores. v2

Sharding: core = (batch b, head-group g), 4 heads per core (DH=256).
Per-core layout is pair-oriented: heads (h0,h1) and (h2,h3) form "pairs";
within a pair, the even head lives on SBUF partitions 0-63 and the odd head
on partitions 64-127.  This enables:
  - row-tiled score matmuls: K=64 matmuls for the two heads of a pair run
    in different PE row-groups (tile_position auto-derived from
    base_partition) and execute concurrently.
  - col-tiled AV matmuls: lhsT = V_h [128j, 64] for the even head targets
    psum partitions 0-63 and the odd head partitions 64-127 of the same
    psum tile; the two K=128/M=64 matmuls run in different PE col-groups
    concurrently at full array utilization.
Softmax Z is NOT computed with a ones-column (that would force M=65);
instead zpart[p, i] += E[p, i] accumulates on the Vector engine per j-tile
and a final ones-vector matmul reduces over partitions.
exp() runs as one N=2048 ACT instruction per (pair, jt) reading a 4-bank
psum score tile [128, 2048] = [h_even 1024-i | h_odd 1024-i].
i is processed in two halves (IH=1024) so AV accumulators fit in 2 psum
banks per pair; out-projection of half 0 overlaps attention of half 1.
"""

import numpy as np

B, S, D, H = 2, 2048, 1024, 16
DK = D // H              # 64
N_CORES = 8
GROUPS = N_CORES // B    # 4 head-groups
DH = D // GROUPS         # 256 per core
H_CORE = DH // DK        # 4 heads per core
SCALE = 1.0 / float(np.sqrt(DK))

P = 128
SC = 512
IH = 1024                # i-half
JT = S // P              # 16 j tiles
KT = D // P              # 8 contraction tiles for projections
NSC = S // SC            # 4
PAIRS = H_CORE // 2      # 2
NOUT = D // P            # 8


def build_nc():
    import concourse.bacc as bacc
    import concourse.mybir as mybir
    import concourse.tile as tile

    f32 = mybir.dt.float32
    bf16 = mybir.dt.bfloat16
    Exp = mybir.ActivationFunctionType.Exp
    cdt = bf16

    nc = bacc.Bacc("TRN2", target_bir_lowering=False, debug=False)

    def mm_noldw(out, lhsT, rhs, start, stop, tile_position=None):
        """Matmul that reuses the stationary already in the PE array
        (InstMatmult with ldweights=False)."""
        te = nc.tensor
        ifmap_ap = te.lower_ap(rhs.opt({0}), opt=False)
        weights_ap = te.lower_ap(lhsT.opt({0}), opt=False,
                                 for_matmul_weights=True)
        out_ap = te.lower_ap(out)

        def round_up(s):
            for v in (32, 64, 128):
                if v >= s:
                    return v

        tile_size = (round_up(rhs.partition_size()),
                     round_up(out.partition_size()))
        if tile_position is None:
            tile_position = (lhsT.base_partition(), out.base_partition())
        return te.add_instruction(mybir.InstMatmult(
            name=nc.get_next_instruction_name(),
            replication_resolution=0,
            replication_shift_amnt=0,
            replication_num_rows=0,
            start_tensor_calc=start,
            stop_tensor_calc=stop,
            ins=[ifmap_ap, weights_ap],
            outs=[out_ap],
            perf_mode=None,
            is_transpose=None,
            ifmap_quant_offset=None,
            weights_quant_offset=None,
            bass_skip_group_check=False,
            tile_position=tile_position,
            tile_size=tile_size,
            ldweights=False,
        ))

    qT = nc.dram_tensor("qT", [D, S], cdt, kind="ExternalInput")
    kTd = nc.dram_tensor("kTd", [D, S], cdt, kind="ExternalInput")
    vT = nc.dram_tensor("vT", [D, S], cdt, kind="ExternalInput")
    wq = nc.dram_tensor("wq", [D, DH], cdt, kind="ExternalInput")
    wk = nc.dram_tensor("wk", [D, DH], cdt, kind="ExternalInput")
    wv = nc.dram_tensor("wv", [D, DH], cdt, kind="ExternalInput")
    wo = nc.dram_tensor("wo", [DH, D], cdt, kind="ExternalInput")
    bq = nc.dram_tensor("bq", [P, PAIRS], f32, kind="ExternalInput")
    bk = nc.dram_tensor("bk", [P, PAIRS], f32, kind="ExternalInput")
    bvb = nc.dram_tensor("bvb", [P, DH], f32, kind="ExternalInput")
    bo = nc.dram_tensor("bo", [P, NOUT], f32, kind="ExternalInput")
    outT = nc.dram_tensor("outT", [D, S], cdt, kind="ExternalOutput")

    with tile.TileContext(nc) as tc:
        with (
            tc.tile_pool(name="const", bufs=1) as cpool,
            tc.tile_pool(name="pers", bufs=1) as pers,
            tc.tile_pool(name="stream", bufs=1) as stream,
            tc.tile_pool(name="psum", bufs=1, space="PSUM") as psum,
            tc.tile_pool(name="dscratch", bufs=1, space="DRAM") as dscratch,
        ):
            # ---- constants ----
            wq_sb = cpool.tile([P, KT, DH], cdt, name="wq_sb")
            wk_sb = cpool.tile([P, KT, DH], cdt, name="wk_sb")
            wv_sb = cpool.tile([P, KT, DH], cdt, name="wv_sb")
            wo_sb = cpool.tile([P, PAIRS, D], cdt, name="wo_sb")
            bq_sb = cpool.tile([P, PAIRS], f32, name="bq_sb")
            bk_sb = cpool.tile([P, PAIRS], f32, name="bk_sb")
            bvb_sb = cpool.tile([P, DH], f32, name="bvb_sb")
            bo_sb = cpool.tile([P, NOUT], f32, name="bo_sb")
            ones_sb = cpool.tile([P, 1], cdt, name="ones_sb")
            nc.sync.dma_start(wq_sb[:], wq[:, :].rearrange("(ko p) n -> p ko n", p=P))
            nc.sync.dma_start(wk_sb[:], wk[:, :].rearrange("(ko p) n -> p ko n", p=P))
            nc.sync.dma_start(wv_sb[:], wv[:, :].rearrange("(ko p) n -> p ko n", p=P))
            nc.sync.dma_start(wo_sb[:], wo[:, :].rearrange("(c p) n -> p c n", p=P))
            nc.sync.dma_start(bq_sb[:], bq[:, :])
            nc.sync.dma_start(bk_sb[:], bk[:, :])
            nc.sync.dma_start(bvb_sb[:], bvb[:, :])
            nc.sync.dma_start(bo_sb[:], bo[:, :])
            nc.vector.memset(ones_sb[:], 1.0)


            # ---- persistent activations ----
            qt_p = [pers.tile([P, S], cdt, name=f"qt{c}") for c in range(PAIRS)]
            kt_p = [pers.tile([P, S], cdt, name=f"kt{c}") for c in range(PAIRS)]
            v_sb = pers.tile([P, JT, DH], cdt, name="v_sb")
            on_p = [pers.tile([P, S], cdt, name=f"on{c}") for c in range(PAIRS)]
            zp = {(c, ih): pers.tile([P, 2 * IH], cdt, name=f"zp{c}_{ih}")
                  for c in range(PAIRS) for ih in range(2)}

            # ---- projection generators ----
            def qk_proj(src, w_sb, b_sb, dst, c, si_list):
                """Project one pair-chunk c for the given si chunks into the
                pair tile dst[c]: psum rows 0-63 = even head dims, 64-127 odd."""
                for si in si_list:
                    ins = []
                    for kt in range(KT):
                        t = stream.tile([P, SC], cdt, tag="instream", bufs=12,
                                        name=f"in_{src.name}_{c}_{si}_{kt}")
                        nc.gpsimd.dma_start(
                            t[:], src[kt * P:(kt + 1) * P, si * SC:(si + 1) * SC])
                        ins.append(t)
                        yield
                    ps = psum.tile([P, SC], f32, tag="mm", bufs=2,
                                   name=f"ps_{src.name}_{c}_{si}")
                    for kt in range(KT):
                        nc.tensor.matmul(
                            ps[:],
                            lhsT=w_sb[:, kt, c * P:(c + 1) * P],
                            rhs=ins[kt][:],
                            start=(kt == 0), stop=(kt == KT - 1))
                        yield
                    nc.vector.tensor_add(
                        dst[:, si * SC:(si + 1) * SC], ps[:],
                        b_sb[:, c:c + 1].to_broadcast((P, SC)))
                    yield

            def v_proj(si_list):
                """V natural: psum[s_sub, dh] accumulated over kt."""
                for si in si_list:
                    ins = []
                    for kt in range(KT):
                        t = stream.tile([P, SC], cdt, tag="instream", bufs=12,
                                        name=f"in_v_{si}_{kt}")
                        nc.sync.dma_start(
                            t[:], vT[kt * P:(kt + 1) * P, si * SC:(si + 1) * SC])
                        ins.append(t)
                        yield
                    for sub in range(SC // P):
                        jt_idx = si * (SC // P) + sub
                        ps = psum.tile([P, DH], f32, tag="mm", bufs=2,
                                       name=f"ps_v_{jt_idx}")
                        for kt in range(KT):
                            nc.tensor.matmul(
                                ps[:],
                                lhsT=ins[kt][:, sub * P:(sub + 1) * P],
                                rhs=wv_sb[:, kt, :],
                                start=(kt == 0), stop=(kt == KT - 1))
                            yield
                        nc.vector.tensor_add(v_sb[:, jt_idx, :], ps[:], bvb_sb[:])
                        yield

            # ---- attention generator for one (pair, ih) ----
            _avs = {}
            _avsb = {}

            def attention(pair, ih):
                i0 = ih * IH
                avs = [psum.tile([P, SC], f32, tag="av", bufs=2,
                                 name=f"av_{pair}_{ih}_{ic}") for ic in range(2)]
                _avs[(pair, ih)] = avs
                for jt in range(JT):
                    e_hh = []
                    for hh in range(2):   # even / odd head of pair
                        b0 = hh * DK      # base partition 0 or 64
                        sc_t = psum.tile([P, IH], f32, tag="sc", bufs=2,
                                         name=f"sc_{pair}_{ih}_{jt}_{hh}")
                        for ic in range(2):
                            nc.tensor.matmul(
                                sc_t[:, ic * SC:(ic + 1) * SC],
                                lhsT=kt_p[pair][b0:b0 + DK, jt * P:(jt + 1) * P],
                                rhs=qt_p[pair][b0:b0 + DK,
                                               i0 + ic * SC:i0 + (ic + 1) * SC],
                                start=True, stop=True)
                        e_t = stream.tile([P, IH], cdt, tag="e", bufs=6,
                                          name=f"e_{pair}_{ih}_{jt}_{hh}")
                        nc.scalar.activation(e_t[:], sc_t[:], Exp,
                                             bias=0.0, scale=SCALE)
                        e_hh.append(e_t)
                    # AV, col-tiled pair: even head -> psum rows 0-63,
                    # odd head -> rows 64-127
                    for ic in range(2):
                        for hh in range(2):
                            h = pair * 2 + hh
                            nc.tensor.matmul(
                                avs[ic][hh * DK:(hh + 1) * DK, :],
                                lhsT=v_sb[:, jt, h * DK:(h + 1) * DK],
                                rhs=e_hh[hh][:, ic * SC:(ic + 1) * SC],
                                start=(jt == 0), stop=(jt == JT - 1))
                    # zpart accumulate on DVE (after AV so AV is not gated)
                    for hh in range(2):
                        if jt == 0:
                            nc.vector.tensor_copy(
                                zp[pair, ih][:, hh * IH:(hh + 1) * IH], e_hh[hh][:])
                        else:
                            nc.vector.tensor_add(
                                zp[pair, ih][:, hh * IH:(hh + 1) * IH],
                                zp[pair, ih][:, hh * IH:(hh + 1) * IH], e_hh[hh][:])
                    yield
                # fast psum release: copy AV accumulators to SBUF f32
                av_sb = stream.tile([P, IH], f32, tag="avsb", bufs=2,
                                    name=f"avsb_{pair}_{ih}")
                for ic in range(2):
                    nc.vector.tensor_copy(av_sb[:, ic * SC:(ic + 1) * SC],
                                          avs[ic][:])
                _avsb[(pair, ih)] = av_sb
                yield

            def attention_fin(pair, ih):
                i0 = ih * IH
                av_sb = _avsb[(pair, ih)]
                # ---- Z: partition-reduce zpart with ones-matmuls ----
                # 4 chunks of 512 -> rows 0,32,64,96 of one psum bank
                zps = psum.tile([P, SC], f32, tag="mm", bufs=2,
                                name=f"zps_{pair}_{ih}")
                for q in range(2 * IH // SC):
                    nc.tensor.matmul(
                        zps[q * 32:q * 32 + 1, :],
                        lhsT=ones_sb[:],
                        rhs=zp[pair, ih][:, q * SC:(q + 1) * SC],
                        start=True, stop=True,
                        tile_position=(0, q * 32))
                    yield
                # reshape z rows [4x512] into [16,128] via DRAM to give the
                # reciprocal 16 lanes of work instead of 1
                zraw = stream.tile([P, SC], f32, tag="zraw", bufs=2,
                                   name=f"zraw_{pair}_{ih}")
                zd_raw = dscratch.tile([4, SC], f32, tag="zdr", bufs=2,
                                       name=f"zdr_{pair}_{ih}")
                for q in range(4):
                    nc.vector.tensor_copy(zraw[q * 32:q * 32 + 1, :],
                                          zps[q * 32:q * 32 + 1, :])
                    nc.sync.dma_start(zd_raw[q:q + 1, :],
                                      zraw[q * 32:q * 32 + 1, :])
                zr = stream.tile([16, P], f32, tag="zr", bufs=2,
                                 name=f"zr_{pair}_{ih}")
                nc.sync.dma_start(
                    zr[:], zd_raw[:, :].rearrange("q (a b) -> (q a) b", b=P))
                rz = stream.tile([16, P], f32, tag="rz", bufs=2,
                                 name=f"rz_{pair}_{ih}")
                nc.vector.reciprocal(rz[:], zr[:])
                zd = dscratch.tile([2, IH], f32, tag="zd", bufs=2,
                                   name=f"zd_{pair}_{ih}")
                for hh in range(2):
                    nc.sync.dma_start(
                        zd[hh:hh + 1, :].rearrange("a (p b) -> (a p) b", b=P),
                        rz[hh * 8:(hh + 1) * 8, :])
                yield
                rzb = stream.tile([P, IH], f32, tag="rzb", bufs=2,
                                  name=f"rzb_{pair}_{ih}")
                for hh in range(2):
                    nc.sync.dma_start(
                        rzb[hh * DK:(hh + 1) * DK, :],
                        zd[hh:hh + 1, :].to_broadcast((DK, IH)))
                # normalize: on_p = av_sb * broadcast(1/Z), bf16
                nc.vector.tensor_mul(on_p[pair][:, i0:i0 + IH], av_sb[:], rzb[:])
                yield

            # ---- output projection for one ih ----
            def oproj(ih):
                i0 = ih * IH
                for n in range(NOUT):
                    for ic in range(2):
                        ps = psum.tile([P, SC], f32, tag="mm", bufs=2,
                                       name=f"ps_o_{ih}_{n}_{ic}")
                        for c in range(PAIRS):
                            nc.tensor.matmul(
                                ps[:],
                                lhsT=wo_sb[:, c, n * P:(n + 1) * P],
                                rhs=on_p[c][:, i0 + ic * SC:i0 + (ic + 1) * SC],
                                start=(c == 0), stop=(c == PAIRS - 1))
                        o_sb = stream.tile([P, SC], cdt, tag="osb", bufs=4,
                                           name=f"o_sb_{ih}_{n}_{ic}")
                        nc.vector.tensor_add(
                            o_sb[:], ps[:],
                            bo_sb[:, n:n + 1].to_broadcast((P, SC)))
                        nc.sync.dma_start(
                            outT[n * P:(n + 1) * P, i0 + ic * SC:i0 + (ic + 1) * SC],
                            o_sb[:])
                        yield

            def run(gen):
                for _ in gen:
                    pass

            def interleave(main, filler, ratio):
                """Step main; after each main step, step filler ratio times."""
                for _ in main:
                    for _ in range(ratio):
                        if filler is not None:
                            if next(filler, StopIteration) is StopIteration:
                                filler = None
                for _ in (filler or ()):
                    pass

            def chain(*gens):
                for g in gens:
                    yield from g

            # Phase 0: minimal prefix for attention(pair0, ih0) jt 0-3
            run(qk_proj(kTd, wk_sb, bk_sb, kt_p[0], 0, (0,)))
            run(qk_proj(qT, wq_sb, bq_sb, qt_p[0], 0, (0, 1)))
            run(v_proj((0,)))

            # P1: attention(pair0, ih0); K-c0/V si chunks feed just ahead of
            # their jt consumption; Q-c0 si23 (needed by P2) fills the rest.
            rest1 = chain(
                qk_proj(kTd, wk_sb, bk_sb, kt_p[0], 0, (1,)),
                v_proj((1,)),
                qk_proj(kTd, wk_sb, bk_sb, kt_p[0], 0, (2,)),
                v_proj((2,)),
                qk_proj(kTd, wk_sb, bk_sb, kt_p[0], 0, (3,)),
                v_proj((3,)),
                qk_proj(qT, wq_sb, bq_sb, qt_p[0], 0, (2, 3)),
            )
            interleave(attention(0, 0), rest1, 20)
            # P2: attention(pair0, ih1); start K/Q for pair1
            rest2 = chain(
                attention_fin(0, 0),
                qk_proj(kTd, wk_sb, bk_sb, kt_p[1], 1, (0,)),
                qk_proj(qT, wq_sb, bq_sb, qt_p[1], 1, (0, 1)),
                qk_proj(kTd, wk_sb, bk_sb, kt_p[1], 1, (1,)),
            )
            interleave(attention(0, 1), rest2, 5)
            # P3: attention(pair1, ih0); K-c1 si2/si3 feed jt8+/jt12+
            rest3 = chain(
                qk_proj(kTd, wk_sb, bk_sb, kt_p[1], 1, (2,)),
                qk_proj(kTd, wk_sb, bk_sb, kt_p[1], 1, (3,)),
                attention_fin(0, 1),
                qk_proj(qT, wq_sb, bq_sb, qt_p[1], 1, (2, 3)),
            )
            interleave(attention(1, 0), rest3, 5)
            # P4: attention(pair1, ih1); pair1-ih0 finalize + oproj(ih0)
            rest4 = chain(attention_fin(1, 0), oproj(0))
            interleave(attention(1, 1), rest4, 2)
            # P5: tail -- keep PE warm through the z-dance (the warmup
            # matmuls become ready when av_sb(1,1) is written), then oproj
            wu2 = psum.tile([P, SC], f32, tag="mm", bufs=2, name="wu2_ps")
            for _ in range(4):
                nc.tensor.matmul(wu2[:], lhsT=_avsb[(1, 1)][:, 0:P],
                                 rhs=_avsb[(1, 1)][:, 0:SC],
                                 start=True, stop=True)
            run(attention_fin(1, 1))
            run(oproj(1))

    nc.finalize()
    return nc


def make_in_maps(query, key, value, Wq, bq, Wk, bk, Wv, bv, Wo, bo):
    import ml_dtypes
    f = lambda a: np.asarray(a, dtype=np.float32)
    cvt = lambda a: np.ascontiguousarray(
        np.asarray(a, np.float32).astype(ml_dtypes.bfloat16))
    query, key, value = f(query), f(key), f(value)
    Wq, Wk, Wv, Wo = f(Wq), f(Wk), f(Wv), f(Wo)
    bq, bk, bv, bo = f(bq), f(bk), f(bv), f(bo)
    in_maps = []
    for core in range(N_CORES):
        b, g = core // GROUPS, core % GROUPS
        sl = slice(g * DH, (g + 1) * DH)
        in_maps.append({
            "qT": cvt(query[b].T),
            "kTd": cvt(key[b].T),
            "vT": cvt(value[b].T),
            "wq": cvt(Wq[:, sl]),
            "wk": cvt(Wk[:, sl]),
            "wv": cvt(Wv[:, sl]),
            "wo": cvt(Wo[sl, :]),
            # pair-chunk bias layout: [128, PAIRS]; partition p of chunk c is
            # head-dim p of pair c (rows 0-63 even head, 64-127 odd head)
            "bq": np.ascontiguousarray(bq[sl].reshape(PAIRS, P).T),
            "bk": np.ascontiguousarray(bk[sl].reshape(PAIRS, P).T),
            "bvb": np.ascontiguousarray(
                np.broadcast_to(bv[sl][None], (P, DH))),
            "bo": (np.ascontiguousarray(bo.reshape(NOUT, P).T)
                   if g == 0 else np.zeros((P, NOUT), np.float32)),
        })
    return in_maps


TRACE = False
LAST_RESULT = None
DTYPE = "bf16"
_NC_CACHE = {}


def kernel(query, key, value, Wq, bq, Wk, bk, Wv, bv, Wo, bo):
    global LAST_RESULT
    from concourse.bass_utils import run_bass_kernel_spmd

    if "nc" not in _NC_CACHE:
        _NC_CACHE["nc"] = build_nc()
    nc = _NC_CACHE["nc"]

    in_maps = make_in_maps(query, key, value, Wq, bq, Wk, bk, Wv, bv, Wo, bo)
    kwargs = {}
    if TRACE:
        kwargs = dict(trace=True, trace_cores=[0])
    res = run_bass_kernel_spmd(nc, in_maps, core_ids=list(range(N_CORES)), **kwargs)
    LAST_RESULT = res

    out = np.zeros((B, S, D), np.float32)
    for core in range(N_CORES):
        b = core // GROUPS
        out[b] += res.results[core]["outT"].T.astype(np.float32)
    return out


# revision 4
# speedup vs baseline: 1.0080x; 1.0076x over previous
"""Multi-head attention, tensor-parallel across 8 Trainium2 NeuronCores. v2

Sharding: core = (batch b, head-group g), 4 heads per core (DH=256).
Per-core layout is pair-oriented: heads (h0,h1) and (h2,h3) form "pairs";
within a pair, the even head lives on SBUF partitions 0-63 and the odd head
on partitions 64-127.  This enables:
  - row-tiled score matmuls: K=64 matmuls for the two heads of a pair run
    in different PE row-groups (tile_position auto-derived from
    base_partition) and execute concurrently.
  - col-tiled AV matmuls: lhsT = V_h [128j, 64] for the even head targets
    psum partitions 0-63 and the odd head partitions 64-127 of the same
    psum tile; the two K=128/M=64 matmuls run in different PE col-groups
    concurrently at full array utilization.
Softmax Z is NOT computed with a ones-column (that would force M=65);
instead zpart[p, i] += E[p, i] accumulates on the Vector engine per j-tile
and a final ones-vector matmul reduces over partitions.
exp() runs as one N=2048 ACT instruction per (pair, jt) reading a 4-bank
psum score tile [128, 2048] = [h_even 1024-i | h_odd 1024-i].
i is processed in two halves (IH=1024) so AV accumulators fit in 2 psum
banks per pair; out-projection of half 0 overlaps attention of half 1.
"""

import numpy as np

B, S, D, H = 2, 2048, 1024, 16
DK = D // H              # 64
N_CORES = 8
GROUPS = N_CORES // B    # 4 head-groups
DH = D // GROUPS         # 256 per core
H_CORE = DH // DK        # 4 heads per core
SCALE = 1.0 / float(np.sqrt(DK))

P = 128
SC = 512
IH = 1024                # i-half
JT = S // P              # 16 j tiles
KT = D // P              # 8 contraction tiles for projections
NSC = S // SC            # 4
PAIRS = H_CORE // 2      # 2
NOUT = D // P            # 8


def build_nc():
    import concourse.bacc as bacc
    import concourse.mybir as mybir
    import concourse.tile as tile

    f32 = mybir.dt.float32
    bf16 = mybir.dt.bfloat16
    Exp = mybir.ActivationFunctionType.Exp
    cdt = bf16

    nc = bacc.Bacc("TRN2", target_bir_lowering=False, debug=False)

    def mm_noldw(out, lhsT, rhs, start, stop, tile_position=None):
        """Matmul that reuses the stationary already in the PE array
        (InstMatmult with ldweights=False)."""
        te = nc.tensor
        ifmap_ap = te.lower_ap(rhs.opt({0}), opt=False)
        weights_ap = te.lower_ap(lhsT.opt({0}), opt=False,
                                 for_matmul_weights=True)
        out_ap = te.lower_ap(out)

        def round_up(s):
            for v in (32, 64, 128):
                if v >= s:
                    return v

        tile_size = (round_up(rhs.partition_size()),
                     round_up(out.partition_size()))
        if tile_position is None:
            tile_position = (lhsT.base_partition(), out.base_partition())
        return te.add_instruction(mybir.InstMatmult(
            name=nc.get_next_instruction_name(),
            replication_resolution=0,
            replication_shift_amnt=0,
            replication_num_rows=0,
            start_tensor_calc=start,
            stop_tensor_calc=stop,
            ins=[ifmap_ap, weights_ap],
            outs=[out_ap],
            perf_mode=None,
            is_transpose=None,
            ifmap_quant_offset=None,
            weights_quant_offset=None,
            bass_skip_group_check=False,
            tile_position=tile_position,
            tile_size=tile_size,
            ldweights=False,
        ))

    qT = nc.dram_tensor("qT", [D, S], cdt, kind="ExternalInput")
    kTd = nc.dram_tensor("kTd", [D, S], cdt, kind="ExternalInput")
    vT = nc.dram_tensor("vT", [D, S], cdt, kind="ExternalInput")
    wq = nc.dram_tensor("wq", [D, DH], cdt, kind="ExternalInput")
    wk = nc.dram_tensor("wk", [D, DH], cdt, kind="ExternalInput")
    wv = nc.dram_tensor("wv", [D, DH], cdt, kind="ExternalInput")
    wo = nc.dram_tensor("wo", [DH, D], cdt, kind="ExternalInput")
    bq = nc.dram_tensor("bq", [P, PAIRS], f32, kind="ExternalInput")
    bk = nc.dram_tensor("bk", [P, PAIRS], f32, kind="ExternalInput")
    bvb = nc.dram_tensor("bvb", [P, DH], f32, kind="ExternalInput")
    bo = nc.dram_tensor("bo", [P, NOUT], f32, kind="ExternalInput")
    outT = nc.dram_tensor("outT", [D, S], cdt, kind="ExternalOutput")

    with tile.TileContext(nc) as tc:
        with (
            tc.tile_pool(name="const", bufs=1) as cpool,
            tc.tile_pool(name="pers", bufs=1) as pers,
            tc.tile_pool(name="stream", bufs=1) as stream,
            tc.tile_pool(name="psum", bufs=1, space="PSUM") as psum,
            tc.tile_pool(name="dscratch", bufs=1, space="DRAM") as dscratch,
        ):
            # ---- constants ----
            wq_sb = cpool.tile([P, KT, DH], cdt, name="wq_sb")
            wk_sb = cpool.tile([P, KT, DH], cdt, name="wk_sb")
            wv_sb = cpool.tile([P, KT, DH], cdt, name="wv_sb")
            wo_sb = cpool.tile([P, PAIRS, D], cdt, name="wo_sb")
            bq_sb = cpool.tile([P, PAIRS], f32, name="bq_sb")
            bk_sb = cpool.tile([P, PAIRS], f32, name="bk_sb")
            bvb_sb = cpool.tile([P, DH], f32, name="bvb_sb")
            bo_sb = cpool.tile([P, NOUT], f32, name="bo_sb")
            ones_sb = cpool.tile([P, 1], cdt, name="ones_sb")
            nc.sync.dma_start(wk_sb[:], wk[:, :].rearrange("(ko p) n -> p ko n", p=P))
            nc.sync.dma_start(wq_sb[:], wq[:, :].rearrange("(ko p) n -> p ko n", p=P))
            nc.sync.dma_start(wv_sb[:], wv[:, :].rearrange("(ko p) n -> p ko n", p=P))
            nc.sync.dma_start(wo_sb[:], wo[:, :].rearrange("(c p) n -> p c n", p=P))
            nc.sync.dma_start(bq_sb[:], bq[:, :])
            nc.sync.dma_start(bk_sb[:], bk[:, :])
            nc.sync.dma_start(bvb_sb[:], bvb[:, :])
            nc.sync.dma_start(bo_sb[:], bo[:, :])
            nc.vector.memset(ones_sb[:], 1.0)


            # ---- persistent activations ----
            qt_p = [pers.tile([P, S], cdt, name=f"qt{c}") for c in range(PAIRS)]
            kt_p = [pers.tile([P, S], cdt, name=f"kt{c}") for c in range(PAIRS)]
            v_sb = pers.tile([P, JT, DH], cdt, name="v_sb")
            on_p = [pers.tile([P, S], cdt, name=f"on{c}") for c in range(PAIRS)]
            zp = {(c, ih): pers.tile([P, 2 * IH], cdt, name=f"zp{c}_{ih}")
                  for c in range(PAIRS) for ih in range(2)}

            # ---- projection generators ----
            def qk_proj(src, w_sb, b_sb, dst, c, si_list):
                """Project one pair-chunk c for the given si chunks into the
                pair tile dst[c]: psum rows 0-63 = even head dims, 64-127 odd."""
                for si in si_list:
                    ins = []
                    for kt in range(KT):
                        t = stream.tile([P, SC], cdt, tag="instream", bufs=12,
                                        name=f"in_{src.name}_{c}_{si}_{kt}")
                        nc.gpsimd.dma_start(
                            t[:], src[kt * P:(kt + 1) * P, si * SC:(si + 1) * SC])
                        ins.append(t)
                        yield
                    ps = psum.tile([P, SC], f32, tag="mm", bufs=2,
                                   name=f"ps_{src.name}_{c}_{si}")
                    for kt in range(KT):
                        nc.tensor.matmul(
                            ps[:],
                            lhsT=w_sb[:, kt, c * P:(c + 1) * P],
                            rhs=ins[kt][:],
                            start=(kt == 0), stop=(kt == KT - 1))
                        yield
                    nc.vector.tensor_add(
                        dst[:, si * SC:(si + 1) * SC], ps[:],
                        b_sb[:, c:c + 1].to_broadcast((P, SC)))
                    yield

            def v_proj(si_list):
                """V natural: psum[s_sub, dh] accumulated over kt."""
                for si in si_list:
                    ins = []
                    for kt in range(KT):
                        t = stream.tile([P, SC], cdt, tag="instream", bufs=12,
                                        name=f"in_v_{si}_{kt}")
                        nc.sync.dma_start(
                            t[:], vT[kt * P:(kt + 1) * P, si * SC:(si + 1) * SC])
                        ins.append(t)
                        yield
                    for sub in range(SC // P):
                        jt_idx = si * (SC // P) + sub
                        ps = psum.tile([P, DH], f32, tag="mm", bufs=2,
                                       name=f"ps_v_{jt_idx}")
                        for kt in range(KT):
                            nc.tensor.matmul(
                                ps[:],
                                lhsT=ins[kt][:, sub * P:(sub + 1) * P],
                                rhs=wv_sb[:, kt, :],
                                start=(kt == 0), stop=(kt == KT - 1))
                            yield
                        nc.vector.tensor_add(v_sb[:, jt_idx, :], ps[:], bvb_sb[:])
                        yield

            # ---- attention generator for one (pair, ih) ----
            _avs = {}
            _avsb = {}

            def attention(pair, ih):
                i0 = ih * IH
                avs = [psum.tile([P, SC], f32, tag="av", bufs=2,
                                 name=f"av_{pair}_{ih}_{ic}") for ic in range(2)]
                _avs[(pair, ih)] = avs
                for jt in range(JT):
                    e_hh = []
                    for hh in range(2):   # even / odd head of pair
                        b0 = hh * DK      # base partition 0 or 64
                        sc_t = psum.tile([P, IH], f32, tag="sc", bufs=2,
                                         name=f"sc_{pair}_{ih}_{jt}_{hh}")
                        for ic in range(2):
                            nc.tensor.matmul(
                                sc_t[:, ic * SC:(ic + 1) * SC],
                                lhsT=kt_p[pair][b0:b0 + DK, jt * P:(jt + 1) * P],
                                rhs=qt_p[pair][b0:b0 + DK,
                                               i0 + ic * SC:i0 + (ic + 1) * SC],
                                start=True, stop=True)
                        e_t = stream.tile([P, IH], cdt, tag="e", bufs=6,
                                          name=f"e_{pair}_{ih}_{jt}_{hh}")
                        nc.scalar.activation(e_t[:], sc_t[:], Exp,
                                             bias=0.0, scale=SCALE)
                        e_hh.append(e_t)
                    # AV, col-tiled pair: even head -> psum rows 0-63,
                    # odd head -> rows 64-127
                    for ic in range(2):
                        for hh in range(2):
                            h = pair * 2 + hh
                            nc.tensor.matmul(
                                avs[ic][hh * DK:(hh + 1) * DK, :],
                                lhsT=v_sb[:, jt, h * DK:(h + 1) * DK],
                                rhs=e_hh[hh][:, ic * SC:(ic + 1) * SC],
                                start=(jt == 0), stop=(jt == JT - 1))
                    # zpart accumulate on DVE (after AV so AV is not gated)
                    for hh in range(2):
                        if jt == 0:
                            nc.vector.tensor_copy(
                                zp[pair, ih][:, hh * IH:(hh + 1) * IH], e_hh[hh][:])
                        else:
                            nc.vector.tensor_add(
                                zp[pair, ih][:, hh * IH:(hh + 1) * IH],
                                zp[pair, ih][:, hh * IH:(hh + 1) * IH], e_hh[hh][:])
                    yield
                # fast psum release: copy AV accumulators to SBUF f32
                av_sb = stream.tile([P, IH], f32, tag="avsb", bufs=2,
                                    name=f"avsb_{pair}_{ih}")
                for ic in range(2):
                    nc.vector.tensor_copy(av_sb[:, ic * SC:(ic + 1) * SC],
                                          avs[ic][:])
                _avsb[(pair, ih)] = av_sb
                yield

            def attention_fin(pair, ih):
                i0 = ih * IH
                av_sb = _avsb[(pair, ih)]
                # ---- Z: partition-reduce zpart with ones-matmuls ----
                # 4 chunks of 512 -> rows 0,32,64,96 of one psum bank
                zps = psum.tile([P, SC], f32, tag="mm", bufs=2,
                                name=f"zps_{pair}_{ih}")
                for q in range(2 * IH // SC):
                    nc.tensor.matmul(
                        zps[q * 32:q * 32 + 1, :],
                        lhsT=ones_sb[:],
                        rhs=zp[pair, ih][:, q * SC:(q + 1) * SC],
                        start=True, stop=True,
                        tile_position=(0, q * 32))
                    yield
                # reshape z rows [4x512] into [16,128] via DRAM to give the
                # reciprocal 16 lanes of work instead of 1
                zraw = stream.tile([P, SC], f32, tag="zraw", bufs=2,
                                   name=f"zraw_{pair}_{ih}")
                zd_raw = dscratch.tile([4, SC], f32, tag="zdr", bufs=2,
                                       name=f"zdr_{pair}_{ih}")
                for q in range(4):
                    nc.vector.tensor_copy(zraw[q * 32:q * 32 + 1, :],
                                          zps[q * 32:q * 32 + 1, :])
                    nc.sync.dma_start(zd_raw[q:q + 1, :],
                                      zraw[q * 32:q * 32 + 1, :])
                zr = stream.tile([16, P], f32, tag="zr", bufs=2,
                                 name=f"zr_{pair}_{ih}")
                nc.sync.dma_start(
                    zr[:], zd_raw[:, :].rearrange("q (a b) -> (q a) b", b=P))
                rz = stream.tile([16, P], f32, tag="rz", bufs=2,
                                 name=f"rz_{pair}_{ih}")
                nc.vector.reciprocal(rz[:], zr[:])
                zd = dscratch.tile([2, IH], f32, tag="zd", bufs=2,
                                   name=f"zd_{pair}_{ih}")
                for hh in range(2):
                    nc.sync.dma_start(
                        zd[hh:hh + 1, :].rearrange("a (p b) -> (a p) b", b=P),
                        rz[hh * 8:(hh + 1) * 8, :])
                yield
                rzb = stream.tile([P, IH], f32, tag="rzb", bufs=2,
                                  name=f"rzb_{pair}_{ih}")
                for hh in range(2):
                    nc.sync.dma_start(
                        rzb[hh * DK:(hh + 1) * DK, :],
                        zd[hh:hh + 1, :].to_broadcast((DK, IH)))
                # normalize: on_p = av_sb * broadcast(1/Z), bf16
                nc.vector.tensor_mul(on_p[pair][:, i0:i0 + IH], av_sb[:], rzb[:])
                yield

            # ---- output projection for one ih ----
            Ident = mybir.ActivationFunctionType.Identity

            def oproj(ih):
                i0 = ih * IH
                for n in range(NOUT):
                    for ic in range(2):
                        idx = n * 2 + ic
                        ps = psum.tile([P, SC], f32, tag="mm", bufs=2,
                                       name=f"ps_o_{ih}_{n}_{ic}")
                        for c in range(PAIRS):
                            nc.tensor.matmul(
                                ps[:],
                                lhsT=wo_sb[:, c, n * P:(n + 1) * P],
                                rhs=on_p[c][:, i0 + ic * SC:i0 + (ic + 1) * SC],
                                start=(c == 0), stop=(c == PAIRS - 1))
                        o_sb = stream.tile([P, SC], cdt, tag="osb", bufs=4,
                                           name=f"o_sb_{ih}_{n}_{ic}")
                        if ih == 1 and idx % 2 == 0:
                            # tail: ACT is idle; split drains across engines
                            nc.scalar.activation(o_sb[:], ps[:], Ident,
                                                 bias=bo_sb[:, n:n + 1],
                                                 scale=1.0)
                        else:
                            nc.vector.tensor_add(
                                o_sb[:], ps[:],
                                bo_sb[:, n:n + 1].to_broadcast((P, SC)))
                        nc.sync.dma_start(
                            outT[n * P:(n + 1) * P, i0 + ic * SC:i0 + (ic + 1) * SC],
                            o_sb[:])
                        yield

            def run(gen):
                for _ in gen:
                    pass

            def interleave(main, filler, ratio):
                """Step main; after each main step, step filler ratio times."""
                for _ in main:
                    for _ in range(ratio):
                        if filler is not None:
                            if next(filler, StopIteration) is StopIteration:
                                filler = None
                for _ in (filler or ()):
                    pass

            def chain(*gens):
                for g in gens:
                    yield from g

            # Phase 0: minimal prefix for attention(pair0, ih0) jt 0-3
            run(qk_proj(kTd, wk_sb, bk_sb, kt_p[0], 0, (0,)))
            run(qk_proj(qT, wq_sb, bq_sb, qt_p[0], 0, (0, 1)))
            run(v_proj((0,)))

            # P1: attention(pair0, ih0); K-c0/V si chunks feed just ahead of
            # their jt consumption; Q-c0 si23 (needed by P2) fills the rest.
            rest1 = chain(
                qk_proj(kTd, wk_sb, bk_sb, kt_p[0], 0, (1,)),
                v_proj((1,)),
                qk_proj(kTd, wk_sb, bk_sb, kt_p[0], 0, (2,)),
                v_proj((2,)),
                qk_proj(kTd, wk_sb, bk_sb, kt_p[0], 0, (3,)),
                v_proj((3,)),
                qk_proj(qT, wq_sb, bq_sb, qt_p[0], 0, (2, 3)),
            )
            interleave(attention(0, 0), rest1, 20)
            # P2: attention(pair0, ih1); start K/Q for pair1
            rest2 = chain(
                attention_fin(0, 0),
                qk_proj(kTd, wk_sb, bk_sb, kt_p[1], 1, (0,)),
                qk_proj(qT, wq_sb, bq_sb, qt_p[1], 1, (0, 1)),
                qk_proj(kTd, wk_sb, bk_sb, kt_p[1], 1, (1,)),
            )
            interleave(attention(0, 1), rest2, 5)
            # P3: attention(pair1, ih0); K-c1 si2/si3 feed jt8+/jt12+
            rest3 = chain(
                qk_proj(kTd, wk_sb, bk_sb, kt_p[1], 1, (2,)),
                qk_proj(kTd, wk_sb, bk_sb, kt_p[1], 1, (3,)),
                attention_fin(0, 1),
                qk_proj(qT, wq_sb, bq_sb, qt_p[1], 1, (2, 3)),
            )
            interleave(attention(1, 0), rest3, 5)
            # P4: attention(pair1, ih1); pair1-ih0 finalize + oproj(ih0)
            rest4 = chain(attention_fin(1, 0), oproj(0))
            interleave(attention(1, 1), rest4, 2)
            # P5: tail -- keep PE warm through the z-dance (the warmup
            # matmuls become ready when av_sb(1,1) is written), then oproj
            wu2 = psum.tile([P, SC], f32, tag="mm", bufs=2, name="wu2_ps")
            for _ in range(4):
                nc.tensor.matmul(wu2[:], lhsT=_avsb[(1, 1)][:, 0:P],
                                 rhs=_avsb[(1, 1)][:, 0:SC],
                                 start=True, stop=True)
            run(attention_fin(1, 1))
            run(oproj(1))

    nc.finalize()
    return nc


def make_in_maps(query, key, value, Wq, bq, Wk, bk, Wv, bv, Wo, bo):
    import ml_dtypes
    f = lambda a: np.asarray(a, dtype=np.float32)
    cvt = lambda a: np.ascontiguousarray(
        np.asarray(a, np.float32).astype(ml_dtypes.bfloat16))
    query, key, value = f(query), f(key), f(value)
    Wq, Wk, Wv, Wo = f(Wq), f(Wk), f(Wv), f(Wo)
    bq, bk, bv, bo = f(bq), f(bk), f(bv), f(bo)
    in_maps = []
    for core in range(N_CORES):
        b, g = core // GROUPS, core % GROUPS
        sl = slice(g * DH, (g + 1) * DH)
        in_maps.append({
            "qT": cvt(query[b].T),
            "kTd": cvt(key[b].T),
            "vT": cvt(value[b].T),
            "wq": cvt(Wq[:, sl]),
            "wk": cvt(Wk[:, sl]),
            "wv": cvt(Wv[:, sl]),
            "wo": cvt(Wo[sl, :]),
            # pair-chunk bias layout: [128, PAIRS]; partition p of chunk c is
            # head-dim p of pair c (rows 0-63 even head, 64-127 odd head)
            "bq": np.ascontiguousarray(bq[sl].reshape(PAIRS, P).T),
            "bk": np.ascontiguousarray(bk[sl].reshape(PAIRS, P).T),
            "bvb": np.ascontiguousarray(
                np.broadcast_to(bv[sl][None], (P, DH))),
            "bo": (np.ascontiguousarray(bo.reshape(NOUT, P).T)
                   if g == 0 else np.zeros((P, NOUT), np.float32)),
        })
    return in_maps


TRACE = False
LAST_RESULT = None
DTYPE = "bf16"
_NC_CACHE = {}


def kernel(query, key, value, Wq, bq, Wk, bk, Wv, bv, Wo, bo):
    global LAST_RESULT
    from concourse.bass_utils import run_bass_kernel_spmd

    if "nc" not in _NC_CACHE:
        _NC_CACHE["nc"] = build_nc()
    nc = _NC_CACHE["nc"]

    in_maps = make_in_maps(query, key, value, Wq, bq, Wk, bk, Wv, bv, Wo, bo)
    kwargs = {}
    if TRACE:
        kwargs = dict(trace=True, trace_cores=[0])
    res = run_bass_kernel_spmd(nc, in_maps, core_ids=list(range(N_CORES)), **kwargs)
    LAST_RESULT = res

    out = np.zeros((B, S, D), np.float32)
    for core in range(N_CORES):
        b = core // GROUPS
        out[b] += res.results[core]["outT"].T.astype(np.float32)
    return out
